# revision 47
# baseline (speedup 1.0000x reference)
"""GQA attention kernel for 8 Trainium2 NeuronCores.

Head-parallel sharding: core c owns q-heads [4c, 4c+4) and kv-head c.
Each core computes its 4 heads' attention and a partial output
projection (row-parallel wo); the host sums the 8 partials (bf16).

Attention processes heads in PAIRS: the even head lives in SBUF
partitions 0-63, the odd head in 64-127.  Score matmuls for the two
heads are row-tiled (tile_position (0,0) / (64,0), K=64 each) so they
run concurrently in the two halves of the PE array; k is duplicated
into both partition halves to feed them.  One fused exp covers both
heads' scores chunk ([128,1024] PSUM -> SBUF).  The AV matmul uses
[v | 1] as the stationary operand so row HD of the accumulator is the
softmax denominator; diagonal chunks trim their masked query columns.
The output projection is interleaved per query block to keep the PE
warm (HAM) and spread the output DMA.
"""

import sys

sys.path.insert(0, "/opt/trn_rl_repo")

import numpy as np
import ml_dtypes

import concourse.bacc as bacc
import concourse.bass as bass
import concourse.mybir as mybir
from concourse import tile
from concourse.bass_utils import run_bass_kernel_spmd
from concourse.masks import make_identity

B, T, D = 2, 2048, 2048
H, HKV, HD = 32, 8, 64
NCORE = 8
HLOC = H // NCORE          # 4 q heads per core
QCOLS = HLOC * HD          # 256
NB = T // 512              # token nblocks per batch
KC = D // 128              # contraction chunks for projections

F32 = mybir.dt.float32
BF16 = mybir.dt.bfloat16
SD = BF16
NPSD = ml_dtypes.bfloat16

SWAP_MASK = [i ^ 1 for i in range(32)]  # adjacent pair swap per quadrant


def build_nc():
    nc = bacc.Bacc(None, target_bir_lowering=False, debug=False)

    xT = nc.dram_tensor("xT", [D, B * T], SD, kind="ExternalInput")
    wq_d = nc.dram_tensor("wq", [D, QCOLS], SD, kind="ExternalInput")
    wkv_d = nc.dram_tensor("wkv", [D, 2 * HD], SD, kind="ExternalInput")
    wo_d = nc.dram_tensor("wo", [QCOLS, D], SD, kind="ExternalInput")
    rAq_d = nc.dram_tensor("ropeAq", [128, T], SD, kind="ExternalInput")
    rBq_d = nc.dram_tensor("ropeBq", [128, T], SD, kind="ExternalInput")
    rAkv_d = nc.dram_tensor("ropeAkv", [128, T], SD, kind="ExternalInput")
    rBkv_d = nc.dram_tensor("ropeBkv", [128, T], SD, kind="ExternalInput")
    out_d = nc.dram_tensor("out", [B * T, D], SD, kind="ExternalOutput")

    from contextlib import ExitStack

    with tile.TileContext(nc) as tc:
        with ExitStack() as es:
            pool_specs = [
                ("consts", 1, None), ("xp", 6, None), ("shufp", 3, None),
                ("ropea", 3, None), ("ropeb", 3, None), ("qpp", 4, None),
                ("k2p", 2, None), ("v64p", 2, None), ("vextp", 2, None),
                ("ptp", 6, None), ("attp", 4, None), ("rcp", 6, None),
                ("rbp", 4, None), ("otp", 3, None),
                ("scp", 2, "PSUM"), ("avp", 2, "PSUM"), ("mp", 2, "PSUM"),
            ]
            pools = {}
            for pname, bufs, space in pool_specs:
                kw = {"name": pname, "bufs": bufs}
                if space:
                    kw["space"] = space
                pools[pname] = es.enter_context(tc.tile_pool(**kw))
            consts = pools["consts"]; xp = pools["xp"]; shufp = pools["shufp"]
            ropea = pools["ropea"]; ropeb = pools["ropeb"]; qpp = pools["qpp"]
            k2p = pools["k2p"]; v64p = pools["v64p"]; vextp = pools["vextp"]
            ptp = pools["ptp"]; attp = pools["attp"]; rcp = pools["rcp"]
            rbp = pools["rbp"]; otp = pools["otp"]
            scp = pools["scp"]; avp = pools["avp"]; mp = pools["mp"]

            # ---- constants ----
            # only the projection weights are needed immediately; the rope
            # tables and wo are DMA'd after the first x tiles (see below) so
            # the first matmul starts as early as possible
            wq_sb = consts.tile([128, KC, QCOLS], SD)
            wkv_sb = consts.tile([128, KC, 2 * HD], SD)
            # first contraction chunks first, so matmul 0 starts ASAP
            for h in range(4):
                ksl = slice(h * (KC // 4), (h + 1) * (KC // 4))
                nc.sync.dma_start(
                    out=wq_sb[:, ksl, :],
                    in_=wq_d.rearrange("(kc p) m -> p kc m", p=128)[:, ksl, :],
                )
                nc.sync.dma_start(
                    out=wkv_sb[:, ksl, :],
                    in_=wkv_d.rearrange("(kc p) m -> p kc m", p=128)[:, ksl, :],
                )
            wo_sb = consts.tile([128, 2, D], SD)
            rAq = consts.tile([128, T], SD)
            rBq = consts.tile([128, T], SD)
            rAkv = consts.tile([128, T], SD)
            rBkv = consts.tile([128, T], SD)

            ident = consts.tile([128, 128], SD)
            make_identity(nc, ident[:])
            # trimask[k, q] = 1.0 if k <= q else 0.0  (keys on partitions)
            trimask = consts.tile([128, 128], SD)
            nc.gpsimd.memset(trimask[:], 1.0)
            nc.gpsimd.affine_select(
                out=trimask[:],
                in_=trimask[:],
                compare_op=mybir.AluOpType.is_ge,
                fill=0.0,
                base=0,
                pattern=[[1, 128]],
                channel_multiplier=-1,
            )
            # duplicated for one-op masking of both heads' [*, 2, 128] block
            trimask2 = consts.tile([128, 2, 128], SD)
            nc.vector.tensor_copy(trimask2[:, 0, :], trimask[:])
            nc.vector.tensor_copy(trimask2[:, 1, :], trimask[:])

            for b in range(B):
                # ---- QKV projections + fused RoPE eviction ----
                qpair = [qpp.tile([128, T], SD, tag="qp", name="qp") for _ in range(2)]
                k2 = k2p.tile([128, T], SD, tag="k2", name="k2")
                v64 = v64p.tile([64, T], SD, tag="v64", name="v64")
                v_ext = vextp.tile([128, KC, HD + 1], SD, tag="vext", name="vext")
                nc.gpsimd.memset(v_ext[:], 1.0)

                for nb in range(NB):
                    qps = scp.tile([128, 2, 512], F32, tag="sc", name="qps")
                    kvps = avp.tile([128, 512], F32, tag="av", name="kvps")
                    for kk in range(KC // 2):
                        xt = xp.tile([128, 2, 512], SD, tag="x", name="x")
                        c0 = b * T + nb * 512
                        nc.sync.dma_start(
                            out=xt[:],
                            in_=xT[
                                kk * 256 : (kk + 1) * 256, c0 : c0 + 512
                            ].rearrange("(two p) n -> p two n", p=128),
                        )
                        for i in range(2):
                            kc = 2 * kk + i
                            for p in range(2):
                                nc.tensor.matmul(
                                    qps[:, p, :],
                                    wq_sb[:, kc, p * 128 : (p + 1) * 128],
                                    xt[:, i, :],
                                    start=(kc == 0),
                                    stop=(kc == KC - 1),
                                )
                            nc.tensor.matmul(
                                kvps[:],
                                wkv_sb[:, kc, :],
                                xt[:, i, :],
                                start=(kc == 0),
                                stop=(kc == KC - 1),
                            )
                    if b == 0 and nb == 0:
                        # bulk const loads ride the Activation HWDGE queue so
                        # they never delay the Sync queue's x-tile stream
                        nc.scalar.dma_start(out=rAkv[:], in_=rAkv_d[:])
                        nc.scalar.dma_start(out=rBkv[:], in_=rBkv_d[:])
                        nc.scalar.dma_start(out=rAq[:], in_=rAq_d[:])
                        nc.scalar.dma_start(out=rBq[:], in_=rBq_d[:])
                        nc.scalar.dma_start(
                            out=wo_sb[:], in_=wo_d.rearrange("(g p) n -> p g n", p=128)
                        )
                    sl = slice(nb * 512, (nb + 1) * 512)
                    # kv eviction first: it feeds the v transposes (PE), so
                    # it must not queue behind the q evictions on DVE
                    tmp = shufp.tile([128, 512], F32, tag="shuf", name="shuf")
                    nc.vector.stream_shuffle(tmp[:], kvps[:], SWAP_MASK)
                    t2 = ropea.tile([128, 512], SD, tag="ra", name="ra")
                    nc.vector.tensor_mul(t2[:], kvps[:], rAkv[:, sl])
                    t3 = ropeb.tile([128, 512], SD, tag="rb", name="rb")
                    nc.vector.tensor_mul(t3[:], tmp[:], rBkv[:, sl])
                    nc.vector.tensor_add(k2[0:64, sl], t2[0:64, :], t3[0:64, :])
                    nc.vector.tensor_add(k2[64:128, sl], t2[0:64, :], t3[0:64, :])
                    nc.vector.tensor_copy(v64[:, sl], kvps[64:128, :])
                    # q eviction: rope per head pair
                    for p in range(2):
                        tmp = shufp.tile([128, 512], F32, tag="shuf", name="shuf")
                        nc.vector.stream_shuffle(tmp[:], qps[:, p, :], SWAP_MASK)
                        t2 = ropea.tile([128, 512], SD, tag="ra", name="ra")
                        nc.vector.tensor_mul(t2[:], qps[:, p, :], rAq[:, sl])
                        t3 = ropeb.tile([128, 512], SD, tag="rb", name="rb")
                        nc.vector.tensor_mul(t3[:], tmp[:], rBq[:, sl])
                        nc.vector.tensor_add(qpair[p][:, sl], t2[:], t3[:])
                    # v transpose into [keys, 1 | 64] chunks for this nb
                    for jj in range(4):
                        j = nb * 4 + jj
                        tp = mp.tile([128, HD], SD, tag="mp", name="tpv")
                        nc.tensor.transpose(
                            tp[:],
                            v64[:, j * 128 : (j + 1) * 128],
                            ident[0:64, 0:64],
                        )
                        nc.vector.tensor_copy(v_ext[:, j, 0:HD], tp[:])

                # ---- attention (2 head pairs) + interleaved wo ----
                attT = [
                    attp.tile([128, T], SD, tag="attT", name="attT") for _ in range(2)
                ]
                for qb in range(NB):
                    nch = 4 * qb + 4
                    q0 = qb * 512
                    for p in range(2):
                        # pair 0 and pair 1 use different PSUM rings so the
                        # next pair's AV never waits on this pair's normalize
                        avpool, avtag = (avp, "av") if p == 0 else (mp, "mp")
                        av = [
                            avpool.tile([128, 512], F32, tag=avtag, name="av")
                            for _ in range(2)
                        ]
                        for j in range(nch):
                            sc = scp.tile([128, 2, 512], F32, tag="sc", name="sc")
                            kcol = slice(j * 128, (j + 1) * 128)
                            nc.tensor.matmul(
                                sc[:, 0, :],
                                k2[0:64, kcol],
                                qpair[p][0:64, q0 : q0 + 512],
                                start=True,
                                stop=True,
                            )
                            nc.tensor.matmul(
                                sc[:, 1, :],
                                k2[64:128, kcol],
                                qpair[p][64:128, q0 : q0 + 512],
                                start=True,
                                stop=True,
                            )
                            pt = ptp.tile([128, 2, 512], SD, tag="pt", name="pt")
                            nc.scalar.activation(
                                pt[:], sc[:], mybir.ActivationFunctionType.Exp
                            )
                            jj = j - 4 * qb
                            mc = 128 * jj if jj >= 0 else 0
                            if jj >= 0:
                                nc.vector.tensor_mul(
                                    pt[:, :, mc : mc + 128],
                                    pt[:, :, mc : mc + 128],
                                    trimask2[:],
                                )
                            for g in range(2):
                                nc.tensor.matmul(
                                    av[g][0 : HD + 1, mc:512],
                                    v_ext[:, j, :],
                                    pt[:, g, mc:512],
                                    start=(j == 0),
                                    stop=(j == nch - 1),
                                )
                        # normalize: row HD of av is the denominator.
                        # (recip_approx_fast mis-reads partition-base-64
                        # inputs, so stage the row at partition 0 first)
                        for g in range(2):
                            dent = rcp.tile([1, 512], F32, tag="dent", name="dent")
                            nc.vector.tensor_copy(dent[:], av[g][HD : HD + 1, :])
                            rc = rcp.tile([1, 512], F32, tag="rc", name="rc")
                            nc.vector.reciprocal_approx_fast(rc[:], dent[:])
                            rbs = rbp.tile([64, 512], F32, tag="rbs", name="rbs")
                            nc.gpsimd.partition_broadcast(rbs[:], rc[:], channels=64)
                            nc.vector.tensor_mul(
                                attT[p][g * 64 : (g + 1) * 64, q0 : q0 + 512],
                                av[g][0:HD, :],
                                rbs[:],
                            )
                    # ---- output projection for this qb's token rows ----
                    for mt in range(qb * 4, qb * 4 + 4):
                        ot = otp.tile([128, NB, 512], SD, tag="ot", name="ot")
                        for nb2 in range(NB):
                            op = mp.tile([128, 512], F32, tag="mp", name="op")
                            for g in range(2):
                                nc.tensor.matmul(
                                    op[:],
                                    attT[g][:, mt * 128 : (mt + 1) * 128],
                                    wo_sb[:, g, nb2 * 512 : (nb2 + 1) * 512],
                                    start=(g == 0),
                                    stop=(g == 1),
                                )
                            nc.vector.tensor_copy(ot[:, nb2, :], op[:])
                        r0 = b * T + mt * 128
                        nc.sync.dma_start(out=out_d[r0 : r0 + 128, :], in_=ot[:])

    nc.compile()
    return nc


_NC = None


def _get_nc():
    global _NC
    if _NC is None:
        _NC = build_nc()
    return _NC


def make_in_maps(x, freqs_cos, freqs_sin, wq, wk, wv, wo):
    x = np.asarray(x, np.float32)
    freqs_cos = np.asarray(freqs_cos, np.float32)
    freqs_sin = np.asarray(freqs_sin, np.float32)
    wq = np.asarray(wq, np.float32)
    wk = np.asarray(wk, np.float32)
    wv = np.asarray(wv, np.float32)
    wo = np.asarray(wo, np.float32)

    xT = np.ascontiguousarray(x.reshape(B * T, D).T.astype(NPSD))

    cosT = freqs_cos.T  # [32, T]
    sinT = freqs_sin.T
    A64 = np.empty((64, T), np.float32)
    A64[0::2] = cosT
    A64[1::2] = cosT
    B64 = np.empty((64, T), np.float32)
    B64[0::2] = -sinT
    B64[1::2] = sinT
    one64 = np.ones((64, T), np.float32)
    zero64 = np.zeros((64, T), np.float32)
    rAq = np.ascontiguousarray(np.concatenate([A64, A64], 0).astype(NPSD))
    rBq = np.ascontiguousarray(np.concatenate([B64, B64], 0).astype(NPSD))
    rAkv = np.ascontiguousarray(np.concatenate([A64, one64], 0).astype(NPSD))
    rBkv = np.ascontiguousarray(np.concatenate([B64, zero64], 0).astype(NPSD))

    scale = np.float32(1.0 / np.sqrt(HD))
    in_maps = []
    for c in range(NCORE):
        wq_c = np.ascontiguousarray(
            (wq[:, c * QCOLS : (c + 1) * QCOLS] * scale).astype(NPSD)
        )
        wkv_c = np.ascontiguousarray(
            np.concatenate(
                [wk[:, c * HD : (c + 1) * HD], wv[:, c * HD : (c + 1) * HD]], 1
            ).astype(NPSD)
        )
        wo_c = np.ascontiguousarray(wo[c * QCOLS : (c + 1) * QCOLS, :].astype(NPSD))
        in_maps.append(
            {
                "xT": xT,
                "wq": wq_c,
                "wkv": wkv_c,
                "wo": wo_c,
                "ropeAq": rAq,
                "ropeBq": rBq,
                "ropeAkv": rAkv,
                "ropeBkv": rBkv,
            }
        )
    return in_maps


def run(in_maps, trace=False, **kwargs):
    nc = _get_nc()
    return run_bass_kernel_spmd(
        nc, in_maps, core_ids=list(range(NCORE)), trace=trace, **kwargs
    )


def kernel(x, freqs_cos, freqs_sin, wq, wk, wv, wo):
    in_maps = make_in_maps(x, freqs_cos, freqs_sin, wq, wk, wv, wo)
    res = run(in_maps)
    total = np.zeros((B * T, D), np.float32)
    for r in res.results:
        total += np.asarray(r["out"], np.float32)
    return total.reshape(B, T, D)


# revision 48
# speedup vs baseline: 1.0189x; 1.0189x over previous
"""GQA attention kernel for 8 Trainium2 NeuronCores.

Head-parallel sharding: core c owns q-heads [4c, 4c+4) and kv-head c.
Each core computes its 4 heads' attention and a partial output
projection (row-parallel wo); the host sums the 8 partials (bf16).

Attention processes heads in PAIRS: the even head lives in SBUF
partitions 0-63, the odd head in 64-127.  Score matmuls for the two
heads are row-tiled (tile_position (0,0) / (64,0), K=64 each) so they
run concurrently in the two halves of the PE array; k is duplicated
into both partition halves to feed them.  One fused exp covers both
heads' scores chunk ([128,1024] PSUM -> SBUF).  The AV matmul uses
[v | 1] as the stationary operand so row HD of the accumulator is the
softmax denominator; diagonal chunks trim their masked query columns.
The output projection is interleaved per query block to keep the PE
warm (HAM) and spread the output DMA.
"""

import sys

sys.path.insert(0, "/opt/trn_rl_repo")

import numpy as np
import ml_dtypes

import concourse.bacc as bacc
import concourse.bass as bass
import concourse.mybir as mybir
from concourse import tile
from concourse.bass_utils import run_bass_kernel_spmd
from concourse.masks import make_identity

B, T, D = 2, 2048, 2048
H, HKV, HD = 32, 8, 64
NCORE = 8
HLOC = H // NCORE          # 4 q heads per core
QCOLS = HLOC * HD          # 256
NB = T // 512              # token nblocks per batch
KC = D // 128              # contraction chunks for projections

F32 = mybir.dt.float32
BF16 = mybir.dt.bfloat16
SD = BF16
NPSD = ml_dtypes.bfloat16

SWAP_MASK = [i ^ 1 for i in range(32)]  # adjacent pair swap per quadrant


def build_nc():
    nc = bacc.Bacc(None, target_bir_lowering=False, debug=False)

    xT = nc.dram_tensor("xT", [D, B * T], SD, kind="ExternalInput")
    wq_d = nc.dram_tensor("wq", [D, QCOLS], SD, kind="ExternalInput")
    wkv_d = nc.dram_tensor("wkv", [D, 2 * HD], SD, kind="ExternalInput")
    wo_d = nc.dram_tensor("wo", [QCOLS, D], SD, kind="ExternalInput")
    rAq_d = nc.dram_tensor("ropeAq", [128, T], SD, kind="ExternalInput")
    rBq_d = nc.dram_tensor("ropeBq", [128, T], SD, kind="ExternalInput")
    rAkv_d = nc.dram_tensor("ropeAkv", [128, T], SD, kind="ExternalInput")
    rBkv_d = nc.dram_tensor("ropeBkv", [128, T], SD, kind="ExternalInput")
    out_d = nc.dram_tensor("out", [B * T, D], SD, kind="ExternalOutput")

    from contextlib import ExitStack

    with tile.TileContext(nc) as tc:
        with ExitStack() as es:
            pool_specs = [
                ("consts", 1, None), ("xp", 6, None), ("shufp", 3, None),
                ("ropea", 3, None), ("ropeb", 3, None), ("qpp", 4, None),
                ("k2p", 2, None), ("v64p", 2, None), ("vextp", 2, None),
                ("ptp", 6, None), ("attp", 4, None), ("rcp", 6, None),
                ("rbp", 4, None), ("otp", 3, None),
                ("scp", 2, "PSUM"), ("avp", 2, "PSUM"), ("mp", 2, "PSUM"),
            ]
            pools = {}
            for pname, bufs, space in pool_specs:
                kw = {"name": pname, "bufs": bufs}
                if space:
                    kw["space"] = space
                pools[pname] = es.enter_context(tc.tile_pool(**kw))
            consts = pools["consts"]; xp = pools["xp"]; shufp = pools["shufp"]
            ropea = pools["ropea"]; ropeb = pools["ropeb"]; qpp = pools["qpp"]
            k2p = pools["k2p"]; v64p = pools["v64p"]; vextp = pools["vextp"]
            ptp = pools["ptp"]; attp = pools["attp"]; rcp = pools["rcp"]
            rbp = pools["rbp"]; otp = pools["otp"]
            scp = pools["scp"]; avp = pools["avp"]; mp = pools["mp"]

            # ---- constants ----
            # only the projection weights are needed immediately; the rope
            # tables and wo are DMA'd after the first x tiles (see below) so
            # the first matmul starts as early as possible
            wq_sb = consts.tile([128, KC, QCOLS], SD)
            nc.sync.dma_start(
                out=wq_sb[:], in_=wq_d.rearrange("(kc p) m -> p kc m", p=128)
            )
            wkv_sb = consts.tile([128, KC, 2 * HD], SD)
            nc.sync.dma_start(
                out=wkv_sb[:], in_=wkv_d.rearrange("(kc p) m -> p kc m", p=128)
            )
            wo_sb = consts.tile([128, 2, D], SD)
            rAq = consts.tile([128, T], SD)
            rBq = consts.tile([128, T], SD)
            rAkv = consts.tile([128, T], SD)
            rBkv = consts.tile([128, T], SD)

            ident = consts.tile([128, 128], SD)
            make_identity(nc, ident[:])
            # trimask[k, q] = 1.0 if k <= q else 0.0  (keys on partitions)
            trimask = consts.tile([128, 128], SD)
            nc.gpsimd.memset(trimask[:], 1.0)
            nc.gpsimd.affine_select(
                out=trimask[:],
                in_=trimask[:],
                compare_op=mybir.AluOpType.is_ge,
                fill=0.0,
                base=0,
                pattern=[[1, 128]],
                channel_multiplier=-1,
            )
            # duplicated for one-op masking of both heads' [*, 2, 128] block
            trimask2 = consts.tile([128, 2, 128], SD)
            nc.vector.tensor_copy(trimask2[:, 0, :], trimask[:])
            nc.vector.tensor_copy(trimask2[:, 1, :], trimask[:])

            for b in range(B):
                # ---- QKV projections + fused RoPE eviction ----
                qpair = [qpp.tile([128, T], SD, tag="qp", name="qp") for _ in range(2)]
                k2 = k2p.tile([128, T], SD, tag="k2", name="k2")
                v64 = v64p.tile([64, T], SD, tag="v64", name="v64")
                v_ext = vextp.tile([128, KC, HD + 1], SD, tag="vext", name="vext")
                nc.gpsimd.memset(v_ext[:], 1.0)

                for nb in range(NB):
                    qps = scp.tile([128, 2, 512], F32, tag="sc", name="qps")
                    kvps = avp.tile([128, 512], F32, tag="av", name="kvps")
                    for kk in range(KC // 2):
                        xt = xp.tile([128, 2, 512], SD, tag="x", name="x")
                        c0 = b * T + nb * 512
                        nc.sync.dma_start(
                            out=xt[:],
                            in_=xT[
                                kk * 256 : (kk + 1) * 256, c0 : c0 + 512
                            ].rearrange("(two p) n -> p two n", p=128),
                        )
                        for i in range(2):
                            kc = 2 * kk + i
                            for p in range(2):
                                nc.tensor.matmul(
                                    qps[:, p, :],
                                    wq_sb[:, kc, p * 128 : (p + 1) * 128],
                                    xt[:, i, :],
                                    start=(kc == 0),
                                    stop=(kc == KC - 1),
                                )
                            nc.tensor.matmul(
                                kvps[:],
                                wkv_sb[:, kc, :],
                                xt[:, i, :],
                                start=(kc == 0),
                                stop=(kc == KC - 1),
                            )
                    if b == 0 and nb == 0:
                        # deferred const loads: queued behind nb0's x tiles,
                        # ready well before their first readers
                        nc.sync.dma_start(out=rAq[:], in_=rAq_d[:])
                        nc.sync.dma_start(out=rBq[:], in_=rBq_d[:])
                        nc.sync.dma_start(out=rAkv[:], in_=rAkv_d[:])
                        nc.sync.dma_start(out=rBkv[:], in_=rBkv_d[:])
                        nc.sync.dma_start(
                            out=wo_sb[:], in_=wo_d.rearrange("(g p) n -> p g n", p=128)
                        )
                    sl = slice(nb * 512, (nb + 1) * 512)
                    # kv eviction first: it feeds the v transposes (PE), so
                    # it must not queue behind the q evictions on DVE
                    tmp = shufp.tile([128, 512], F32, tag="shuf", name="shuf")
                    nc.vector.stream_shuffle(tmp[:], kvps[:], SWAP_MASK)
                    t2 = ropea.tile([128, 512], SD, tag="ra", name="ra")
                    nc.vector.tensor_mul(t2[:], kvps[:], rAkv[:, sl])
                    t3 = ropeb.tile([128, 512], SD, tag="rb", name="rb")
                    nc.vector.tensor_mul(t3[:], tmp[:], rBkv[:, sl])
                    nc.vector.tensor_add(k2[0:64, sl], t2[0:64, :], t3[0:64, :])
                    nc.vector.tensor_add(k2[64:128, sl], t2[0:64, :], t3[0:64, :])
                    nc.vector.tensor_copy(v64[:, sl], kvps[64:128, :])
                    # q eviction: rope per head pair
                    for p in range(2):
                        tmp = shufp.tile([128, 512], F32, tag="shuf", name="shuf")
                        nc.vector.stream_shuffle(tmp[:], qps[:, p, :], SWAP_MASK)
                        t2 = ropea.tile([128, 512], SD, tag="ra", name="ra")
                        nc.vector.tensor_mul(t2[:], qps[:, p, :], rAq[:, sl])
                        t3 = ropeb.tile([128, 512], SD, tag="rb", name="rb")
                        nc.vector.tensor_mul(t3[:], tmp[:], rBq[:, sl])
                        nc.vector.tensor_add(qpair[p][:, sl], t2[:], t3[:])
                    # v transpose into [keys, 1 | 64] chunks for this nb
                    for jj in range(4):
                        j = nb * 4 + jj
                        tp = mp.tile([128, HD], SD, tag="mp", name="tpv")
                        nc.tensor.transpose(
                            tp[:],
                            v64[:, j * 128 : (j + 1) * 128],
                            ident[0:64, 0:64],
                        )
                        nc.vector.tensor_copy(v_ext[:, j, 0:HD], tp[:])

                # ---- attention (2 head pairs) + interleaved wo ----
                attT = [
                    attp.tile([128, T], SD, tag="attT", name="attT") for _ in range(2)
                ]
                for qb in range(NB):
                    nch = 4 * qb + 4
                    q0 = qb * 512
                    for p in range(2):
                        # pair 0 and pair 1 use different PSUM rings so the
                        # next pair's AV never waits on this pair's normalize
                        avpool, avtag = (avp, "av") if p == 0 else (mp, "mp")
                        av = [
                            avpool.tile([128, 512], F32, tag=avtag, name="av")
                            for _ in range(2)
                        ]
                        for j in range(nch):
                            sc = scp.tile([128, 2, 512], F32, tag="sc", name="sc")
                            kcol = slice(j * 128, (j + 1) * 128)
                            nc.tensor.matmul(
                                sc[:, 0, :],
                                k2[0:64, kcol],
                                qpair[p][0:64, q0 : q0 + 512],
                                start=True,
                                stop=True,
                            )
                            nc.tensor.matmul(
                                sc[:, 1, :],
                                k2[64:128, kcol],
                                qpair[p][64:128, q0 : q0 + 512],
                                start=True,
                                stop=True,
                            )
                            pt = ptp.tile([128, 2, 512], SD, tag="pt", name="pt")
                            nc.scalar.activation(
                                pt[:], sc[:], mybir.ActivationFunctionType.Exp
                            )
                            jj = j - 4 * qb
                            mc = 128 * jj if jj >= 0 else 0
                            if jj >= 0:
                                nc.vector.tensor_mul(
                                    pt[:, :, mc : mc + 128],
                                    pt[:, :, mc : mc + 128],
                                    trimask2[:],
                                )
                            for g in range(2):
                                nc.tensor.matmul(
                                    av[g][0 : HD + 1, mc:512],
                                    v_ext[:, j, :],
                                    pt[:, g, mc:512],
                                    start=(j == 0),
                                    stop=(j == nch - 1),
                                )
                        # normalize: row HD of av is the denominator.
                        # (recip_approx_fast mis-reads partition-base-64
                        # inputs, so stage the row at partition 0 first)
                        for g in range(2):
                            dent = rcp.tile([1, 512], F32, tag="dent", name="dent")
                            nc.vector.tensor_copy(dent[:], av[g][HD : HD + 1, :])
                            rc = rcp.tile([1, 512], F32, tag="rc", name="rc")
                            nc.vector.reciprocal_approx_fast(rc[:], dent[:])
                            rbs = rbp.tile([64, 512], F32, tag="rbs", name="rbs")
                            nc.gpsimd.partition_broadcast(rbs[:], rc[:], channels=64)
                            nc.vector.tensor_mul(
                                attT[p][g * 64 : (g + 1) * 64, q0 : q0 + 512],
                                av[g][0:HD, :],
                                rbs[:],
                            )
                    # ---- output projection for this qb's token rows ----
                    for mt in range(qb * 4, qb * 4 + 4):
                        ot = otp.tile([128, NB, 512], SD, tag="ot", name="ot")
                        for nb2 in range(NB):
                            op = mp.tile([128, 512], F32, tag="mp", name="op")
                            for g in range(2):
                                nc.tensor.matmul(
                                    op[:],
                                    attT[g][:, mt * 128 : (mt + 1) * 128],
                                    wo_sb[:, g, nb2 * 512 : (nb2 + 1) * 512],
                                    start=(g == 0),
                                    stop=(g == 1),
                                )
                            nc.vector.tensor_copy(ot[:, nb2, :], op[:])
                        r0 = b * T + mt * 128
                        nc.sync.dma_start(out=out_d[r0 : r0 + 128, :], in_=ot[:])

    nc.compile()
    return nc


_NC = None


def _get_nc():
    global _NC
    if _NC is None:
        _NC = build_nc()
    return _NC


def make_in_maps(x, freqs_cos, freqs_sin, wq, wk, wv, wo):
    x = np.asarray(x, np.float32)
    freqs_cos = np.asarray(freqs_cos, np.float32)
    freqs_sin = np.asarray(freqs_sin, np.float32)
    wq = np.asarray(wq, np.float32)
    wk = np.asarray(wk, np.float32)
    wv = np.asarray(wv, np.float32)
    wo = np.asarray(wo, np.float32)

    xT = np.ascontiguousarray(x.reshape(B * T, D).T.astype(NPSD))

    cosT = freqs_cos.T  # [32, T]
    sinT = freqs_sin.T
    A64 = np.empty((64, T), np.float32)
    A64[0::2] = cosT
    A64[1::2] = cosT
    B64 = np.empty((64, T), np.float32)
    B64[0::2] = -sinT
    B64[1::2] = sinT
    one64 = np.ones((64, T), np.float32)
    zero64 = np.zeros((64, T), np.float32)
    rAq = np.ascontiguousarray(np.concatenate([A64, A64], 0).astype(NPSD))
    rBq = np.ascontiguousarray(np.concatenate([B64, B64], 0).astype(NPSD))
    rAkv = np.ascontiguousarray(np.concatenate([A64, one64], 0).astype(NPSD))
    rBkv = np.ascontiguousarray(np.concatenate([B64, zero64], 0).astype(NPSD))

    scale = np.float32(1.0 / np.sqrt(HD))
    in_maps = []
    for c in range(NCORE):
        wq_c = np.ascontiguousarray(
            (wq[:, c * QCOLS : (c + 1) * QCOLS] * scale).astype(NPSD)
        )
        wkv_c = np.ascontiguousarray(
            np.concatenate(
                [wk[:, c * HD : (c + 1) * HD], wv[:, c * HD : (c + 1) * HD]], 1
            ).astype(NPSD)
        )
        wo_c = np.ascontiguousarray(wo[c * QCOLS : (c + 1) * QCOLS, :].astype(NPSD))
        in_maps.append(
            {
                "xT": xT,
                "wq": wq_c,
                "wkv": wkv_c,
                "wo": wo_c,
                "ropeAq": rAq,
                "ropeBq": rBq,
                "ropeAkv": rAkv,
                "ropeBkv": rBkv,
            }
        )
    return in_maps


def run(in_maps, trace=False, **kwargs):
    nc = _get_nc()
    return run_bass_kernel_spmd(
        nc, in_maps, core_ids=list(range(NCORE)), trace=trace, **kwargs
    )


def kernel(x, freqs_cos, freqs_sin, wq, wk, wv, wo):
    in_maps = make_in_maps(x, freqs_cos, freqs_sin, wq, wk, wv, wo)
    res = run(in_maps)
    total = np.zeros((B * T, D), np.float32)
    for r in res.results:
        total += np.asarray(r["out"], np.float32)
    return total.reshape(B, T, D)
